# revision 11
# baseline (speedup 1.0000x reference)
"""Trainium2 Bass kernel for LoopABMIL (attention-based MIL pooling).

reference:
    h = silu(x @ Wp + bp)            # [B, N, H]
    a = h @ Wa[:, 0] + ba            # [B, N]
    p = softmax(a masked to lengths) # [B, N]
    pooled = p @ h                   # [B, H]
    logits = pooled @ Wc + bc        # [B, C]

Strategy: softmax-pooling is associative, so each of the 8 cores processes an
equal number of 128-patch chunks from EVERY bag (round-robin over the bag's
valid chunks only — patches beyond lengths[b] are never loaded or computed).
Each core emits per-bag partials (sum_p e^{a_p} * h_p, sum_p e^{a_p}); the
host merges partials across cores and applies the tiny classifier.  ba cancels
in the softmax ratio and is dropped on device.

Device layout per chunk (patches on PSUM/SBUF partitions):
  h_pre[128p, 256] = sum_k x_chunk_T[k*128:(k+1)*128, p].T @ Wp[k] + 1.T @ bp
  h = silu(h_pre)                       (ScalarE, PSUM -> SBUF)
  a[128p, 1] = reduce_add(h * Wa_bcast) (VectorE tensor_tensor_reduce)
  w = exp(a + mask)                     (ScalarE, mask = 0 or -30000)
  pool[1, 0:256]  += w.T @ h            (TensorE, PSUM accumulate over chunks)
  pool[1, 256:257] += w.T @ ones        (TensorE)
"""

import sys

if "/opt/trn_rl_repo" not in sys.path:
    sys.path.insert(0, "/opt/trn_rl_repo")

from contextlib import ExitStack

import ml_dtypes
import numpy as np

import concourse.bacc as bacc
import concourse.tile as tile
from concourse import mybir
from concourse.bass_utils import run_bass_kernel_spmd

B, N, D, H, C = 8, 8192, 1024, 256, 2
P = 128          # patch chunk size (SBUF partitions)
NCORES = 8
KT = D // P      # k-tiles in the projection contraction
NEG = -30000.0   # additive mask: exp(a + NEG) == 0.0 exactly in f32

BF = mybir.dt.bfloat16
F32 = mybir.dt.float32

_cache: dict = {}


def _build(
    G: int, n_per_bag: tuple, act=None, pool_bf16: bool = True
) -> "bacc.Bacc":
    """One SPMD program shared by all 8 cores: G chunks grouped by bag."""
    if act is None:
        act = mybir.ActivationFunctionType.Silu
    PDT = BF if pool_bf16 else F32
    nc = bacc.Bacc("TRN2", target_bir_lowering=False)

    xpk = nc.dram_tensor("xpk", [G, P, D], BF, kind="ExternalInput")
    maskT = nc.dram_tensor("maskT", [P, G], F32, kind="ExternalInput")
    wp = nc.dram_tensor("wp", [KT, P, H], BF, kind="ExternalInput")
    bprow = nc.dram_tensor("bprow", [1, H], BF, kind="ExternalInput")
    wab = nc.dram_tensor("wab", [P, H], F32, kind="ExternalInput")
    out = nc.dram_tensor("out", [1, B * (H + 1)], F32, kind="ExternalOutput")

    with tile.TileContext(nc) as tc, ExitStack() as ctx:
        const = ctx.enter_context(tc.tile_pool(name="const", bufs=1))
        xpool = ctx.enter_context(tc.tile_pool(name="xp", bufs=4))
        hprep = ctx.enter_context(tc.tile_pool(name="hpre", bufs=2, space="PSUM"))
        hpool = ctx.enter_context(tc.tile_pool(name="hp", bufs=3))
        scrp = ctx.enter_context(tc.tile_pool(name="scr", bufs=2))
        smallp = ctx.enter_context(tc.tile_pool(name="small", bufs=4))
        poolp = ctx.enter_context(tc.tile_pool(name="poolps", bufs=2, space="PSUM"))
        sump = ctx.enter_context(tc.tile_pool(name="sumps", bufs=2, space="PSUM"))
        outp = ctx.enter_context(tc.tile_pool(name="outp", bufs=1))

        wp_t = []
        for k in range(KT):
            t = const.tile([P, H], BF, tag=f"wp{k}")
            nc.sync.dma_start(out=t, in_=wp[k])
            wp_t.append(t)
        bp_t = const.tile([1, H], BF, tag="bp")
        nc.sync.dma_start(out=bp_t, in_=bprow[:])
        wab_t = const.tile([P, H], F32, tag="wab")
        nc.sync.dma_start(out=wab_t, in_=wab[:])
        mask_t = const.tile([P, G], F32, tag="mask")
        nc.sync.dma_start(out=mask_t, in_=maskT[:])
        ones_w = const.tile([1, P], BF, tag="onesw")   # bias-broadcast lhsT
        nc.vector.memset(ones_w, 1.0)
        ones_n = const.tile([P, 1], PDT, tag="onesn")  # denominator rhs
        nc.vector.memset(ones_n, 1.0)
        out_sb = outp.tile([1, B * (H + 1)], F32, tag="outsb")

        g = 0
        for b in range(B):
            nb = n_per_bag[b]
            pool_t = poolp.tile([1, H], F32, tag="pool")
            sum_t = sump.tile([1, 1], F32, tag="sum")
            for j in range(nb):
                xt = xpool.tile([P, D], BF, tag="xt")
                nc.sync.dma_start(out=xt, in_=xpk[g])

                hp = hprep.tile([P, H], F32, tag="hp")
                for k in range(KT):
                    nc.tensor.matmul(
                        hp,
                        lhsT=xt[:, k * P:(k + 1) * P],
                        rhs=wp_t[k],
                        start=(k == 0),
                        stop=False,
                    )
                nc.tensor.matmul(hp, lhsT=ones_w, rhs=bp_t, start=False, stop=True)

                h = hpool.tile([P, H], F32, tag="h")
                nc.scalar.activation(out=h, in_=hp, func=act)
                if pool_bf16:
                    hb = hpool.tile([P, H], BF, tag="hb")
                    nc.scalar.activation(
                        out=hb, in_=hp, func=act
                    )
                else:
                    hb = h

                scr = scrp.tile([P, H], F32, tag="scr")
                a = smallp.tile([P, 1], F32, tag="a")
                nc.vector.tensor_mul(scr, h, wab_t)
                nc.vector.reduce_sum(out=a, in_=scr, axis=mybir.AxisListType.X)

                w = smallp.tile([P, 1], PDT, tag="w")
                nc.scalar.activation(
                    out=w,
                    in_=a,
                    func=mybir.ActivationFunctionType.Exp,
                    bias=mask_t[:, g:g + 1],
                    scale=1.0,
                )

                first, last = (j == 0), (j == nb - 1)
                nc.tensor.matmul(
                    pool_t[0:1, 0:H], lhsT=w, rhs=hb, start=first, stop=last
                )
                nc.tensor.matmul(
                    sum_t[0:1, 0:1], lhsT=w, rhs=ones_n, start=first, stop=last
                )
                g += 1

            nc.scalar.activation(
                out=out_sb[0:1, b * (H + 1):b * (H + 1) + H],
                in_=pool_t,
                func=mybir.ActivationFunctionType.Copy,
            )
            nc.scalar.activation(
                out=out_sb[0:1, b * (H + 1) + H:(b + 1) * (H + 1)],
                in_=sum_t,
                func=mybir.ActivationFunctionType.Copy,
            )

        nc.sync.dma_start(out=out[:], in_=out_sb)

    nc.compile()
    return nc


def _plan(lengths: np.ndarray):
    """Chunk counts: bag b has T_b valid chunks; every core gets n_b slots."""
    lens = np.asarray(lengths, dtype=np.int64)
    T = np.maximum((lens + P - 1) // P, 1)       # valid chunks per bag
    n = (T + NCORES - 1) // NCORES               # per-core slots per bag
    G = int(n.sum())
    return T, n, G


def _pack(x, lengths, T, n, G):
    """Per-core inputs: xpk [G,128,1024] bf16 (lhsT layout) + maskT [128,G]."""
    lens = np.asarray(lengths, dtype=np.int64)
    # x[b, t*128+p, k*128+d] -> xr[b, t, d, k*128+p]  (d = within-k-tile index)
    xr = (
        np.asarray(x)
        .astype(ml_dtypes.bfloat16)
        .reshape(B, N // P, P, KT, P)
        .transpose(0, 1, 4, 3, 2)
        .reshape(B, N // P, P, D)
    )
    in_maps = []
    masks = []
    for c in range(NCORES):
        bs = np.repeat(np.arange(B), n)
        js = np.concatenate([np.arange(nb) for nb in n])
        ts = c + NCORES * js                       # global chunk id per slot
        ts_clip = np.minimum(ts, T[bs] - 1)
        xpk = xr[bs, ts_clip]                      # [G, 128, 1024] bf16
        # valid patches in slot: clip(len - t*128, 0, 128); dummies get 0
        valid = np.clip(lens[bs] - ts * P, 0, P)
        valid[ts >= T[bs]] = 0
        maskT = np.where(
            np.arange(P)[:, None] < valid[None, :], 0.0, NEG
        ).astype(np.float32)
        in_maps.append({"xpk": np.ascontiguousarray(xpk), "maskT": maskT})
        masks.append(maskT)
    return in_maps


def _run(inputs: dict, trace: bool = False):
    x = np.asarray(inputs["x"], dtype=np.float32)
    lengths = np.asarray(inputs["lengths"])
    Wp = np.asarray(inputs["Wp"], dtype=np.float32)
    bp = np.asarray(inputs["bp"], dtype=np.float32)
    Wa = np.asarray(inputs["Wa"], dtype=np.float32)
    Wc = np.asarray(inputs["Wc"], dtype=np.float32)
    ba = np.asarray(inputs["ba"], dtype=np.float32)
    bc = np.asarray(inputs["bc"], dtype=np.float32)

    T, n, G = _plan(lengths)
    key = (G, tuple(int(v) for v in n))
    if key not in _cache:
        _cache[key] = _build(G, key[1])
    nc = _cache[key]

    in_maps = _pack(x, lengths, T, n, G)
    wp_tiles = Wp.reshape(KT, P, H).astype(ml_dtypes.bfloat16)
    bprow = bp.reshape(1, H).astype(ml_dtypes.bfloat16)
    wab = np.tile(Wa[:, 0][None, :], (P, 1)).astype(np.float32)
    for m in in_maps:
        m["wp"] = wp_tiles
        m["bprow"] = bprow
        m["wab"] = wab

    res = run_bass_kernel_spmd(
        nc, in_maps, core_ids=list(range(NCORES)), trace=trace
    )

    parts = np.stack(
        [r["out"].reshape(B, H + 1) for r in res.results]
    ).astype(np.float64)                            # [cores, B, H+1]
    v = parts[:, :, :H].sum(axis=0)                 # [B, H]
    s = parts[:, :, H].sum(axis=0)                  # [B]
    pooled = v / s[:, None]
    logits = pooled @ Wc.astype(np.float64) + bc.astype(np.float64)
    return logits.astype(np.float32), res.exec_time_ns


def kernel(**inputs) -> np.ndarray:
    logits, _ = _run(inputs, trace=False)
    return logits
